# revision 1
# baseline (speedup 1.0000x reference)
"""Trainium2 Bass kernel for nn_EnergyFunction (dense transformer block).

Reference math (B=2, S=2048, D=1024, H=8 heads, hd=128):
    K  = x @ Wk.T            [B,S,D] -> heads [B,H,S,hd]
    V  = x @ Wv.T
    E  = (K K^T)/sqrt(hd)    per head, causal mask (q >= k allowed)
    P  = softmax(-E, axis=k)
    O  = P @ V               -> [B,S,D]
    out = (O + x @ Wself.T) @ Wout.T

Sharding (8 cores): core c -> batch b=c//4, head pair hp=c%4 (heads 2hp,2hp+1,
dims ds=[256*hp, 256*hp+256)).  Each core computes
    partial_c = (O_heads + x @ Wself.T[:,ds]) @ Wout.T[ds,:]   [S, D]
and the host sums the 4 partials per batch (row/column-parallel Wout split).

On-core layout trick: all attention tensors are kept "transposed"
(k or head-dim on partitions, q on free dim).  E is symmetric, so score
tiles are computed directly in (k-part, q-free) orientation by swapping
matmul operands -- no on-chip transposes are needed anywhere.  The softmax
denominator l_q = sum_k P[q,k] is taken with an all-ones [128,128] matmul
accumulated alongside the P@V matmuls, which lands 1/l's operand in PSUM
already broadcast across partitions (no cross-partition reduction or
transpose is ever needed).  Softmax max-subtraction
is skipped: |E|/sqrt(hd) <= ~11 for this distribution, exp() is safe in f32.

Matmuls run in float32r (full PE rate at free-dim>=256, ~1.5e-4 rel err).
The 1/sqrt(hd) scaling is folded into Wk on the host (hd**-0.25 on both
operands of K K^T).
"""

import os
import sys

import numpy as np

if "/opt/trn_rl_repo" not in sys.path:
    sys.path.insert(0, "/opt/trn_rl_repo")

import concourse.bass as bass
import concourse.mybir as mybir
import concourse.tile as tile
from concourse.bass import ts
from concourse.bass_utils import run_bass_kernel_spmd

B, S, D = 2, 2048, 1024
H = 8
HD = D // H          # 128 head dim
HPC = 2              # heads per core
DS = HPC * HD        # 256 dims per core
N_CORES = 8
P = 128              # partitions
QC = 512             # q chunk width
NQC = S // QC        # 4 q chunks
NKT = S // P         # 16 k tiles
NDC = D // P         # 8 contraction chunks over D

F32 = mybir.dt.float32
F32R = mybir.dt.float32r
BF16 = mybir.dt.bfloat16
EXP = mybir.ActivationFunctionType.Exp


def _legalize_waits(nc):
    """This toolchain's walrus rejects >1 semaphore wait on several
    instruction structs (Drain/CTRL allows none, Matmult/Ldweights S3_LW
    allows one).  Hoist excess waits onto same-engine NOPs placed
    immediately before the offending instruction."""
    for blk in nc.main_func.blocks:
        insts = blk.instructions
        new = []
        changed = False
        for ins in insts:
            si = ins.sync_info
            if si is not None and si.on_wait:
                allow = 0 if ins.opcode == "Drain" else 1
                waits = list(si.on_wait)
                if len(waits) > allow:
                    cut = len(waits) - allow
                    for k, w in enumerate(waits[:cut]):
                        nop = mybir.InstNoOp(
                            name=f"{ins.name}-wsplit{k}", engine=ins.engine
                        )
                        nop.sync_info = mybir.SyncInfo(on_wait=[w], on_update=[])
                        new.append(nop)
                    ins.sync_info = mybir.SyncInfo(
                        on_wait=waits[cut:], on_update=list(si.on_update)
                    )
                    changed = True
            new.append(ins)
        if changed:
            blk.instructions = new


def _build(repeats=1, loop_n=None, copy_eng="mix", skip_l=False, phases="BCDEF", xt8=False, esub=True, exp_dve=False, no_mask=False, d_mode="sym"):
    """loop_n: timing-only mode — wrap the body in a device-side For_i loop
    so NEFF execution time dominates the ~200 ms axon RPC floor.
    copy_eng/skip_l/phases: timing experiment knobs (skip_l and partial
    phases produce WRONG results — timing only)."""
    nc = bass.Bass()

    xT = nc.dram_tensor("xT", [D, S], F32R, kind="ExternalInput")
    wkT = nc.dram_tensor("wkT", [D, DS], F32R, kind="ExternalInput")
    wvT = nc.dram_tensor("wvT", [D, DS], F32R, kind="ExternalInput")
    wselfT = nc.dram_tensor("wselfT", [D, DS], F32R, kind="ExternalInput")
    woutT = nc.dram_tensor("woutT", [DS, D], F32R, kind="ExternalInput")
    ones_m = nc.dram_tensor("ones_m", [P, P], BF16, kind="ExternalInput")
    ones_r = nc.dram_tensor("ones_r", [P, P], F32R, kind="ExternalInput")
    mask01 = nc.dram_tensor("mask01", [P, P], BF16, kind="ExternalInput")
    mask01t = nc.dram_tensor("mask01t", [P, P], BF16, kind="ExternalInput")
    part = nc.dram_tensor("part", [S, D], F32, kind="ExternalOutput")
    # tiny completion-marker output: lets timing harnesses wait for NEFF
    # completion without pulling the 8 MB partial off the device
    tick = nc.dram_tensor("tick", [1, 8], F32, kind="ExternalOutput")
    dbg = (
        nc.dram_tensor("dbg", [P, 4 * QC], BF16, kind="ExternalOutput")
        if d_mode == "cl"
        else None
    )

    with tile.TileContext(nc) as tc:
        with (
            tc.tile_pool(name="persist", bufs=1) as pp,
            tc.tile_pool(name="pt_pool", bufs=(8 if d_mode == "sym" else 1)) as pt_pool,
            tc.tile_pool(name="ptr_pool", bufs=(1 if d_mode == "sym" else 2)) as ptr_pool,
            tc.tile_pool(name="strip_pool", bufs=(1 if d_mode == "sym" else 36)) as strip_pool,
            tc.tile_pool(name="lp_pool", bufs=(1 if d_mode == "sym" else 6)) as lp_pool,
            tc.tile_pool(name="rb_pool", bufs=2) as rb_pool,
            tc.tile_pool(name="out_pool", bufs=3) as out_pool,
            tc.tile_pool(name="ps_a", bufs=(4 if d_mode == "sym" else 6), space="PSUM") as ps_a,
            tc.tile_pool(name="ps_ot", bufs=2, space="PSUM") as ps_ot,
            tc.tile_pool(name="ps_l", bufs=(2 if d_mode == "sym" else 0) or 1, space="PSUM") as ps_l,
        ):
            # ---- persistent SBUF tensors ----
            xT_sb = pp.tile([P, NDC, S], F32R, name="xT_sb")
            wkT_sb = pp.tile([P, NDC, DS], F32R, name="wkT_sb")
            wvT_sb = pp.tile([P, NDC, DS], F32R, name="wvT_sb")
            wselfT_sb = pp.tile([P, NDC, DS], F32R, name="wselfT_sb")
            woutT_sb = pp.tile([P, HPC, D], F32R, name="woutT_sb")
            kt_sb = pp.tile([P, HPC, S], BF16, name="kt_sb")
            v_sb = pp.tile([P, NKT, DS], BF16, name="v_sb")
            ut_sb = pp.tile([P, HPC, S], F32R, name="ut_sb")
            onesm_sb = pp.tile([P, P], BF16, name="onesm_sb")
            onesr_sb = pp.tile([P, P], F32R, name="onesr_sb")
            mask_sb = pp.tile([P, P], BF16, name="mask_sb")
            maskt_sb = pp.tile([P, P], BF16, name="maskt_sb")

            def pcopy(dst, src_, i=[0]):
                # psum->sbuf copies: engine per experiment knob
                if copy_eng == "act":
                    nc.scalar.copy(dst, src_)
                elif copy_eng == "dve":
                    nc.vector.tensor_copy(dst, src_)
                else:  # split halves across ACT+DVE so the PSUM bank frees
                    # in half the latency
                    w = dst.shape[-1]
                    h = w // 2
                    nc.scalar.copy(dst[:, :h], src_[:, :h])
                    nc.vector.tensor_copy(dst[:, h:], src_[:, h:])

            # ---- load small consts + weights (per-chunk, in first-use order)
            nc.sync.dma_start(onesm_sb[:], ones_m[:])
            nc.sync.dma_start(onesr_sb[:], ones_r[:])
            nc.sync.dma_start(mask_sb[:], mask01[:])
            nc.sync.dma_start(maskt_sb[:], mask01t[:])
            for c in range(NDC):
                nc.sync.dma_start(wkT_sb[:, c, :], wkT[ts(c, P), :])
            # repeats>1 is a timing-only mode: the compute body is unrolled so
            # per-exec HW time can be measured as a slope, amortizing the
            # ~80ms axon dispatch floor.
            import contextlib

            loop_ctx = (
                tc.For_i(0, loop_n, 1) if loop_n else contextlib.nullcontext()
            )
            with loop_ctx:
              for _rep in range(repeats):
                if xt8:
                    for c in range(NDC):
                        nc.sync.dma_start(xT_sb[:, c, :], xT[ts(c, P), :])
                    for c in range(NDC):
                        nc.sync.dma_start(wvT_sb[:, c, :], wvT[ts(c, P), :])
                else:
                  for j in range(NQC):
                    for c in range(NDC):
                        nc.sync.dma_start(
                            xT_sb[:, c, ts(j, QC)], xT[ts(c, P), ts(j, QC)]
                        )
                    if j == 0:
                        for c in range(NDC):
                            nc.sync.dma_start(wvT_sb[:, c, :], wvT[ts(c, P), :])
                if _rep == 0 or True:
                    for c in range(NDC):
                        nc.sync.dma_start(
                            wselfT_sb[:, c, :], wselfT[ts(c, P), :]
                        )
                    for c in range(HPC):
                        nc.sync.dma_start(
                            woutT_sb[:, c, :], woutT[ts(c, P), :]
                        )

                # ---- phase B: KT[h] = (Wk_h * hd^-0.25) @ x.T  [hd, S] ----
                for h in range(HPC if "B" in phases else 0):
                    for j in range(NQC):
                        ps = ps_a.tile([P, QC], F32, name="ps_b", tag="ps_a")
                        for c in range(NDC):
                            nc.tensor.matmul(
                                ps[:],
                                wkT_sb[:, c, ts(h, HD)],
                                xT_sb[:, c, ts(j, QC)],
                                start=(c == 0),
                                stop=(c == NDC - 1),
                            )
                        pcopy(kt_sb[:, h, ts(j, QC)], ps[:])

                # ---- phase C: V natural [S, 256] ----
                for st in range(NKT if "C" in phases else 0):
                    ps = ps_a.tile([P, QC], F32, name="ps_c", tag="ps_a")
                    for c in range(NDC):
                        nc.tensor.matmul(
                            ps[:, :DS],
                            xT_sb[:, c, ts(st, P)],
                            wvT_sb[:, c, :],
                            start=(c == 0),
                            stop=(c == NDC - 1),
                        )
                    pcopy(v_sb[:, st, :], ps[:, :DS])

                # ---- phase D (classic): q-on-partitions rowblocks, free
                # accum_out denominators, bf16 DMA-transpose of normalized P,
                # no denominator matmuls at all ----
                if d_mode == "cl" and "D" in phases:
                    for j in range(NQC):
                        strips = {}
                        for qt in range(4 * j, 4 * j + 4):
                            for h in range(HPC):
                                klen = (qt + 1) * P
                                nch = (qt // 4) + 1
                                row = ptr_pool.tile(
                                    [P, S], BF16, name="row", tag="row"
                                )
                                lacc = (
                                    lp_pool.tile(
                                        [P, 8], F32, name="lacc", tag="lacc"
                                    )
                                    if not skip_l
                                    else None
                                )
                                nslot = 0
                                for c in range(nch):
                                    w = min(QC, klen - QC * c)
                                    ep = ps_a.tile(
                                        [P, QC], F32, name="epc", tag="ps_a"
                                    )
                                    nc.tensor.matmul(
                                        ep[:, :w],
                                        kt_sb[:, h, ts(qt, P)],
                                        kt_sb[:, h, QC * c : QC * c + w],
                                        start=True,
                                        stop=True,
                                    )
                                    diag = c == nch - 1
                                    if diag and w > P:
                                        nc.scalar.activation(
                                            row[:, QC * c : QC * c + w - P],
                                            ep[:, : w - P],
                                            EXP,
                                            scale=-1.0,
                                            accum_out=(
                                                lacc[:, nslot : nslot + 1]
                                                if lacc is not None else None
                                            ),
                                        )
                                        nslot += 1
                                    elif not diag:
                                        nc.scalar.activation(
                                            row[:, QC * c : QC * c + w],
                                            ep[:, :w],
                                            EXP,
                                            scale=-1.0,
                                            accum_out=(
                                                lacc[:, nslot : nslot + 1]
                                                if lacc is not None else None
                                            ),
                                        )
                                        nslot += 1
                                    if diag:
                                        # diagonal subtile: exp, mask (tril in
                                        # q-part layout), sum via stt accum
                                        dsl = slice(klen - P, klen)
                                        nc.scalar.activation(
                                            row[:, dsl],
                                            ep[:, w - P : w],
                                            EXP,
                                            scale=-1.0,
                                        )
                                        if not no_mask:
                                            nc.vector.scalar_tensor_tensor(
                                                row[:, dsl],
                                                row[:, dsl],
                                                1.0,
                                                maskt_sb[:],
                                                op0=mybir.AluOpType.mult,
                                                op1=mybir.AluOpType.mult,
                                                accum_out=lacc[:, nslot : nslot + 1],
                                            )
                                            nslot += 1
                                if not skip_l:
                                    lsum = lp_pool.tile(
                                        [P, 1], F32, name="lsum", tag="lsum"
                                    )
                                    nc.vector.reduce_sum(
                                        lsum[:],
                                        lacc[:, :nslot],
                                        axis=mybir.AxisListType.X,
                                    )
                                    rv = lp_pool.tile(
                                        [P, 1], F32, name="rvq", tag="rvq"
                                    )
                                    nc.vector.reciprocal(rv[:], lsum[:])
                                # normalize into a fresh tile: dma_start_transpose
                                # must see a clean RAW dep, not an in-place
                                # rewrite of its source
                                row2 = ptr_pool.tile(
                                    [P, S], BF16, name="row2", tag="row2"
                                )
                                if skip_l:
                                    nc.vector.tensor_copy(
                                        row2[:, :klen], row[:, :klen]
                                    )
                                else:
                                    nc.vector.tensor_scalar_mul(
                                        row2[:, :klen], row[:, :klen], rv[:]
                                    )
                                # transpose normalized P tiles into k-part strips
                                for kt in range(qt + 1):
                                    key = (h, kt)
                                    if key not in strips:
                                        strips[key] = strip_pool.tile(
                                            [P, QC], BF16,
                                            name=f"st{h}_{kt}", tag="strip",
                                        )
                                    nc.sync.dma_start_transpose(
                                        strips[key][:, (qt % 4) * P : (qt % 4 + 1) * P],
                                        row2[:, ts(kt, P)],
                                    )
                        if dbg is not None and j == 0:
                            for kt in range(4):
                                nc.sync.dma_start(
                                    dbg[:, ts(kt, QC)], strips[(0, kt)][:]
                                )
                        for h in range(HPC):
                            ot = ps_ot.tile([P, QC], F32, name=f"otc{h}", tag="ps_ot")
                            nkt = 4 * j + 4
                            for kt in range(nkt):
                                c0 = max(0, P * kt - QC * j)
                                nc.tensor.matmul(
                                    ot[:, c0:],
                                    v_sb[:, kt, ts(h, HD)],
                                    strips[(h, kt)][:, c0:],
                                    start=(kt == 0),
                                    stop=(kt == nkt - 1),
                                )
                            pcopy(ut_sb[:, h, ts(j, QC)], ot[:])

                # ---- phase D: attention, heads interleaved per q-chunk ----
                # L matmul uses an all-ones [128,128] lhsT so the softmax
                # denominator lands in PSUM already broadcast across
                # partitions: PE never has to wait on DVE in this phase.
                for j in range(NQC if ("D" in phases and d_mode == "sym") else 0):
                    nkt = 4 * j + 4  # causal: k tiles 0..4j+3
                    ot = {}
                    lb = {}
                    for h in range(HPC):
                        ot[h] = ps_ot.tile(
                            [P, QC], F32, name=f"ot{h}", tag="ps_ot"
                        )
                        lb[h] = ps_l.tile(
                            [P, QC], F32, name=f"lb{h}", tag="ps_l"
                        )
                    # L-matmuls are grouped: PT tiles are pre-summed in f32
                    # on DVE (groups of LG) so PE pays one denominator matmul
                    # per group instead of per k-tile.
                    LG = 1
                    ngrp = {h: 0 for h in range(HPC)}
                    grp_pt = {h: [] for h in range(HPC)}
                    n_l_groups = (nkt + LG - 1) // LG
                    for kt in range(nkt):
                        c0 = max(0, P * kt - QC * j)
                        for h in range(HPC):
                            ep = ps_a.tile([P, QC], F32, name="ep", tag="ps_a")
                            # scores (k-part, q-free): E^T = KT[kt].T @ KT[qch]
                            nc.tensor.matmul(
                                ep[:, c0:],
                                kt_sb[:, h, ts(kt, P)],
                                kt_sb[:, h, QC * j + c0 : QC * (j + 1)],
                                start=True,
                                stop=True,
                            )
                            pt = pt_pool.tile([P, QC], BF16, name="pt", tag="pt")
                            if exp_dve:  # timing-only: wrong numerics
                                nc.vector.tensor_copy(pt[:, c0:], ep[:, c0:])
                            else:
                                nc.scalar.activation(
                                    pt[:, c0:], ep[:, c0:], EXP, scale=-1.0
                                )
                            if (kt >= 4 * j) and not no_mask:
                                # diagonal subtile: zero disallowed (q < k)
                                nc.vector.tensor_mul(
                                    pt[:, c0 : c0 + P],
                                    pt[:, c0 : c0 + P],
                                    mask_sb[:],
                                )
                            nc.tensor.matmul(
                                ot[h][:, c0:],
                                v_sb[:, kt, ts(h, HD)],
                                pt[:, c0:],
                                start=(kt == 0),
                                stop=(kt == nkt - 1),
                            )
                            if not skip_l:
                                grp_pt[h].append((pt, c0))
                                last = kt == nkt - 1
                                if len(grp_pt[h]) == LG or last:
                                    gi = ngrp[h]
                                    tiles = grp_pt[h]
                                    if len(tiles) == 1:
                                        t0, g0 = tiles[0]
                                        nc.tensor.matmul(
                                            lb[h][:, g0:],
                                            onesm_sb[:],
                                            t0[:, g0:],
                                            start=(gi == 0),
                                            stop=last,
                                        )
                                    else:
                                        sp = rb_pool.tile(
                                            [P, QC], F32R, name="sp", tag="sp"
                                        )
                                        (t0, g0), (t1, g1) = tiles[0], tiles[1]
                                        nc.vector.tensor_add(
                                            sp[:, g0:], t0[:, g0:], t1[:, g1:]
                                        ) if g0 == g1 else None
                                        if g0 != g1:
                                            nc.vector.tensor_copy(
                                                sp[:, g0:], t0[:, g0:]
                                            )
                                            nc.vector.tensor_add(
                                                sp[:, g1:],
                                                sp[:, g1:].bitcast(F32),
                                                t1[:, g1:],
                                            )
                                        for tn, gn in tiles[2:]:
                                            nc.vector.tensor_add(
                                                sp[:, gn:],
                                                sp[:, gn:].bitcast(F32),
                                                tn[:, gn:],
                                            )
                                        g0 = tiles[0][1]
                                        nc.tensor.matmul(
                                            lb[h][:, g0:],
                                            onesr_sb[:],
                                            sp[:, g0:],
                                            start=(gi == 0),
                                            stop=last,
                                        )
                                    ngrp[h] += 1
                                    grp_pt[h] = []
                    for h in range(HPC):
                        if skip_l:
                            nc.vector.tensor_copy(
                                ut_sb[:, h, ts(j, QC)], ot[h][:]
                            )
                        else:
                            li = rb_pool.tile([P, QC], F32, name="li", tag="li")
                            nc.vector.reciprocal(li[:], lb[h][:])
                            for sub in range(4):
                                nc.vector.tensor_mul(
                                    ut_sb[:, h, QC * j + P * sub : QC * j + P * (sub + 1)],
                                    ot[h][:, ts(sub, P)],
                                    li[:, ts(sub, P)],
                                )

                # ---- phase E: UT += Wself_h @ x.T ----
                for m in range(HPC if "E" in phases else 0):
                    for j in range(NQC):
                        ps = ps_a.tile([P, QC], F32, name="ps_e2", tag="ps_a")
                        for c in range(NDC):
                            nc.tensor.matmul(
                                ps[:],
                                wselfT_sb[:, c, ts(m, HD)],
                                xT_sb[:, c, ts(j, QC)],
                                start=(c == 0),
                                stop=(c == NDC - 1),
                            )
                        for sub in range(4 if esub else 1):
                            w_ = P if esub else QC
                            sl = slice(QC * j + w_ * sub, QC * j + w_ * (sub + 1))
                            nc.vector.tensor_add(
                                ut_sb[:, m, sl],
                                ut_sb[:, m, sl].bitcast(F32),
                                ps[:, bass.ts(sub, w_)],
                            )

                # ---- phase F: partial = U @ Wout.T slice ----
                for qt in range(NKT if "F" in phases else 0):
                    ob = out_pool.tile([P, D], F32, name="ob", tag="ob")
                    for nch in range(2):
                        ps = ps_a.tile([P, QC], F32, name="ps_f", tag="ps_a")
                        for m in range(HPC):
                            nc.tensor.matmul(
                                ps[:],
                                ut_sb[:, m, ts(qt, P)],
                                woutT_sb[:, m, ts(nch, QC)],
                                start=(m == 0),
                                stop=(m == HPC - 1),
                            )
                        if nch == 0:
                            nc.scalar.copy(ob[:, ts(nch, QC)], ps[:])
                        else:
                            nc.vector.tensor_copy(ob[:, ts(nch, QC)], ps[:])
                    nc.sync.dma_start(part[ts(qt, P), :], ob[:])
                    if qt == NKT - 1:
                        nc.sync.dma_start(tick[:, :], ob[0:1, 0:8])

    _legalize_waits(nc)
    return nc


_NC = None
# attention implementation: "sym" (symmetric-score orientation + denominator
# matmuls) or "cl" (classic orientation + accum_out + bf16 DMA transposes)
D_MODE = os.environ.get("ENERGY_KERNEL_D_MODE", "sym")


def _get_nc():
    global _NC
    if _NC is None:
        _NC = _build(d_mode=D_MODE)
    return _NC


def kernel(x, Wk, Wv, Wself, Wout):
    x = np.ascontiguousarray(np.asarray(x, dtype=np.float32))
    Wk = np.asarray(Wk, dtype=np.float32)
    Wv = np.asarray(Wv, dtype=np.float32)
    Wself = np.asarray(Wself, dtype=np.float32)
    Wout = np.asarray(Wout, dtype=np.float32)

    nc = _get_nc()

    kscale = np.float32(HD ** -0.25)
    xT = [np.ascontiguousarray(x[b].T) for b in range(B)]
    import ml_dtypes

    ones_m = np.ones((P, P), ml_dtypes.bfloat16)
    ones_r = np.ones((P, P), np.float32)
    mask01 = np.triu(np.ones((P, P), ml_dtypes.bfloat16))  # (k,q): allow q >= k
    mask01t = np.tril(np.ones((P, P), ml_dtypes.bfloat16))  # (q,k): allow k <= q


    in_maps = []
    for c in range(N_CORES):
        b, hp = divmod(c, 4)
        ds = slice(DS * hp, DS * (hp + 1))
        in_maps.append(
            {
                "xT": xT[b],
                "wkT": np.ascontiguousarray((Wk[ds, :] * kscale).T),
                "wvT": np.ascontiguousarray(Wv[ds, :].T),
                "wselfT": np.ascontiguousarray(Wself[ds, :].T),
                "woutT": np.ascontiguousarray(Wout[:, ds].T),
                "ones_m": ones_m,
                "ones_r": ones_r,
                "mask01": mask01,
                "mask01t": mask01t,
            }
        )

    res = run_bass_kernel_spmd(nc, in_maps, core_ids=list(range(N_CORES)))

    out = np.empty((B, S, D), np.float32)
    for b in range(B):
        acc = np.zeros((S, D), np.float64)
        for hp in range(4):
            acc += res.results[4 * b + hp]["part"]
        out[b] = acc.astype(np.float32)
    return out



# revision 2
# speedup vs baseline: 2.0483x; 2.0483x over previous
"""Trainium2 Bass kernel for nn_EnergyFunction (dense transformer block).

Reference math (B=2, S=2048, D=1024, H=8 heads, hd=128):
    K  = x @ Wk.T            [B,S,D] -> heads [B,H,S,hd]
    V  = x @ Wv.T
    E  = (K K^T)/sqrt(hd)    per head, causal mask (q >= k allowed)
    P  = softmax(-E, axis=k)
    O  = P @ V               -> [B,S,D]
    out = (O + x @ Wself.T) @ Wout.T

Sharding (8 cores): core c -> batch b=c//4, head pair hp=c%4 (heads 2hp,2hp+1,
dims ds=[256*hp, 256*hp+256)).  Each core computes
    partial_c = (O_heads + x @ Wself.T[:,ds]) @ Wout.T[ds,:]   [S, D]
and the host sums the 4 partials per batch (row/column-parallel Wout split).

On-core layout trick: all attention tensors are kept "transposed"
(k or head-dim on partitions, q on free dim).  E is symmetric, so score
tiles are computed directly in (k-part, q-free) orientation by swapping
matmul operands -- no on-chip transposes are needed anywhere.  The softmax
denominator l_q = sum_k P[q,k] is taken with an all-ones [128,128] matmul
accumulated alongside the P@V matmuls, which lands 1/l's operand in PSUM
already broadcast across partitions (no cross-partition reduction or
transpose is ever needed).  Softmax max-subtraction
is skipped: |E|/sqrt(hd) <= ~11 for this distribution, exp() is safe in f32.

All operands (x, weights, K, V, P, U, partial output) are bf16: halves every
DMA and SBUF footprint, runs matmuls at full PE rate at ANY free size (f32r
needs free>=256), and keeps PSUM accumulation in f32.  rel-err budget is
2e-2; bf16 I/O lands ~2e-3.  The 1/sqrt(hd) scaling is folded into Wk on
the host (hd**-0.25 on both operands of K K^T).
"""

import os
import sys

import numpy as np

if "/opt/trn_rl_repo" not in sys.path:
    sys.path.insert(0, "/opt/trn_rl_repo")

import concourse.bass as bass
import concourse.mybir as mybir
import concourse.tile as tile
from concourse.bass import ts
from concourse.bass_utils import run_bass_kernel_spmd

B, S, D = 2, 2048, 1024
H = 8
HD = D // H          # 128 head dim
HPC = 2              # heads per core
DS = HPC * HD        # 256 dims per core
N_CORES = 8
P = 128              # partitions
QC = 512             # q chunk width
NQC = S // QC        # 4 q chunks
NKT = S // P         # 16 k tiles
NDC = D // P         # 8 contraction chunks over D

F32 = mybir.dt.float32
F32R = mybir.dt.float32r
BF16 = mybir.dt.bfloat16
EXP = mybir.ActivationFunctionType.Exp


def _legalize_waits(nc):
    """This toolchain's walrus rejects >1 semaphore wait on several
    instruction structs (Drain/CTRL allows none, Matmult/Ldweights S3_LW
    allows one).  Hoist excess waits onto same-engine NOPs placed
    immediately before the offending instruction."""
    for blk in nc.main_func.blocks:
        insts = blk.instructions
        new = []
        changed = False
        for ins in insts:
            si = ins.sync_info
            if si is not None and si.on_wait:
                allow = 0 if ins.opcode == "Drain" else 1
                waits = list(si.on_wait)
                if len(waits) > allow:
                    cut = len(waits) - allow
                    for k, w in enumerate(waits[:cut]):
                        nop = mybir.InstNoOp(
                            name=f"{ins.name}-wsplit{k}", engine=ins.engine
                        )
                        nop.sync_info = mybir.SyncInfo(on_wait=[w], on_update=[])
                        new.append(nop)
                    ins.sync_info = mybir.SyncInfo(
                        on_wait=waits[cut:], on_update=list(si.on_update)
                    )
                    changed = True
            new.append(ins)
        if changed:
            blk.instructions = new


def _build(repeats=1, loop_n=None, copy_eng="mix", skip_l=False, phases="BCDEF"):
    """loop_n: timing-only mode — wrap the body in a device-side For_i loop
    so NEFF execution time dominates the ~200 ms axon RPC floor.
    copy_eng/skip_l/phases: timing experiment knobs (skip_l and partial
    phases produce WRONG results — timing only)."""
    nc = bass.Bass()

    xT = nc.dram_tensor("xT", [D, S], BF16, kind="ExternalInput")
    wkT = nc.dram_tensor("wkT", [D, DS], BF16, kind="ExternalInput")
    wvT = nc.dram_tensor("wvT", [D, DS], BF16, kind="ExternalInput")
    wselfT = nc.dram_tensor("wselfT", [D, DS], BF16, kind="ExternalInput")
    woutT = nc.dram_tensor("woutT", [DS, D], BF16, kind="ExternalInput")
    ones_m = nc.dram_tensor("ones_m", [P, P], BF16, kind="ExternalInput")
    mask01 = nc.dram_tensor("mask01", [P, P], BF16, kind="ExternalInput")
    part = nc.dram_tensor("part", [S, D], BF16, kind="ExternalOutput")
    # tiny completion-marker output: lets timing harnesses wait for NEFF
    # completion without pulling the partial off the device
    tick = nc.dram_tensor("tick", [1, 8], BF16, kind="ExternalOutput")

    with tile.TileContext(nc) as tc:
        with (
            tc.tile_pool(name="persist", bufs=1) as pp,
            tc.tile_pool(name="pt_pool", bufs=8) as pt_pool,
            tc.tile_pool(name="rb_pool", bufs=2) as rb_pool,
            tc.tile_pool(name="out_pool", bufs=3) as out_pool,
            tc.tile_pool(name="ps_a", bufs=4, space="PSUM") as ps_a,
            tc.tile_pool(name="ps_ot", bufs=2, space="PSUM") as ps_ot,
            tc.tile_pool(name="ps_l", bufs=2, space="PSUM") as ps_l,
        ):
            # ---- persistent SBUF tensors ----
            xT_sb = pp.tile([P, NDC, S], BF16, name="xT_sb")
            wkT_sb = pp.tile([P, NDC, DS], BF16, name="wkT_sb")
            wvT_sb = pp.tile([P, NDC, DS], BF16, name="wvT_sb")
            wselfT_sb = pp.tile([P, NDC, DS], BF16, name="wselfT_sb")
            woutT_sb = pp.tile([P, HPC, D], BF16, name="woutT_sb")
            kt_sb = pp.tile([P, HPC, S], BF16, name="kt_sb")
            v_sb = pp.tile([P, NKT, DS], BF16, name="v_sb")
            ut_sb = pp.tile([P, HPC, S], BF16, name="ut_sb")
            onesm_sb = pp.tile([P, P], BF16, name="onesm_sb")
            mask_sb = pp.tile([P, P], BF16, name="mask_sb")

            def pcopy(dst, src_):
                # psum->sbuf copies: split halves across ACT+DVE so the
                # PSUM bank frees in half the latency
                if copy_eng == "act":
                    nc.scalar.copy(dst, src_)
                elif copy_eng == "dve":
                    nc.vector.tensor_copy(dst, src_)
                else:
                    w = dst.shape[-1]
                    h = w // 2
                    nc.scalar.copy(dst[:, :h], src_[:, :h])
                    nc.vector.tensor_copy(dst[:, h:], src_[:, h:])

            # ---- load small consts + weights (in first-use order;
            # wkT/x chunk loads are interleaved so phase B starts early)
            nc.sync.dma_start(onesm_sb[:], ones_m[:])
            nc.sync.dma_start(mask_sb[:], mask01[:])
            import contextlib

            loop_ctx = (
                tc.For_i(0, loop_n, 1) if loop_n else contextlib.nullcontext()
            )
            with loop_ctx:
              for _rep in range(repeats):
                for c in range(NDC):
                    nc.sync.dma_start(wkT_sb[:, c, :], wkT[ts(c, P), :])
                    nc.sync.dma_start(
                        xT_sb[:, c, 0:QC], xT[ts(c, P), 0:QC]
                    )
                for j in range(1, NQC):
                    for c in range(NDC):
                        nc.sync.dma_start(
                            xT_sb[:, c, ts(j, QC)], xT[ts(c, P), ts(j, QC)]
                        )
                    if j == 1:
                        for c in range(NDC):
                            nc.sync.dma_start(wvT_sb[:, c, :], wvT[ts(c, P), :])
                for c in range(NDC):
                    nc.sync.dma_start(wselfT_sb[:, c, :], wselfT[ts(c, P), :])
                for c in range(HPC):
                    nc.sync.dma_start(woutT_sb[:, c, :], woutT[ts(c, P), :])

                # ---- phase B: KT[h] = (Wk_h * hd^-0.25) @ x.T  [hd, S] ----
                for h in range(HPC if "B" in phases else 0):
                    for j in range(NQC):
                        ps = ps_a.tile([P, QC], F32, name="ps_b", tag="ps_a")
                        for c in range(NDC):
                            nc.tensor.matmul(
                                ps[:],
                                wkT_sb[:, c, ts(h, HD)],
                                xT_sb[:, c, ts(j, QC)],
                                start=(c == 0),
                                stop=(c == NDC - 1),
                            )
                        pcopy(kt_sb[:, h, ts(j, QC)], ps[:])

                # ---- phase C: V natural [S, 256] ----
                for st in range(NKT if "C" in phases else 0):
                    ps = ps_a.tile([P, QC], F32, name="ps_c", tag="ps_a")
                    for c in range(NDC):
                        nc.tensor.matmul(
                            ps[:, :DS],
                            xT_sb[:, c, ts(st, P)],
                            wvT_sb[:, c, :],
                            start=(c == 0),
                            stop=(c == NDC - 1),
                        )
                    pcopy(v_sb[:, st, :], ps[:, :DS])

                # ---- phase D: attention, heads interleaved per q-chunk ----
                # L matmul uses an all-ones [128,128] lhsT so the softmax
                # denominator lands in PSUM already broadcast across
                # partitions: PE never has to wait on DVE in this phase.
                for j in range(NQC if "D" in phases else 0):
                    nkt = 4 * j + 4  # causal: k tiles 0..4j+3
                    ot = {}
                    lb = {}
                    for h in range(HPC):
                        ot[h] = ps_ot.tile(
                            [P, QC], F32, name=f"ot{h}", tag="ps_ot"
                        )
                        lb[h] = ps_l.tile(
                            [P, QC], F32, name=f"lb{h}", tag="ps_l"
                        )
                    for kt in range(nkt):
                        c0 = max(0, P * kt - QC * j)
                        for h in range(HPC):
                            ep = ps_a.tile([P, QC], F32, name="ep", tag="ps_a")
                            # scores (k-part, q-free): E^T = KT[kt].T @ KT[qch]
                            nc.tensor.matmul(
                                ep[:, c0:],
                                kt_sb[:, h, ts(kt, P)],
                                kt_sb[:, h, QC * j + c0 : QC * (j + 1)],
                                start=True,
                                stop=True,
                            )
                            pt = pt_pool.tile([P, QC], BF16, name="pt", tag="pt")
                            nc.scalar.activation(
                                pt[:, c0:], ep[:, c0:], EXP, scale=-1.0
                            )
                            if kt >= 4 * j:
                                # diagonal subtile: zero disallowed (q < k)
                                nc.vector.tensor_mul(
                                    pt[:, c0 : c0 + P],
                                    pt[:, c0 : c0 + P],
                                    mask_sb[:],
                                )
                            nc.tensor.matmul(
                                ot[h][:, c0:],
                                v_sb[:, kt, ts(h, HD)],
                                pt[:, c0:],
                                start=(kt == 0),
                                stop=(kt == nkt - 1),
                            )
                            if not skip_l:
                                nc.tensor.matmul(
                                    lb[h][:, c0:],
                                    onesm_sb[:],
                                    pt[:, c0:],
                                    start=(kt == 0),
                                    stop=(kt == nkt - 1),
                                )
                    for h in range(HPC):
                        if skip_l:
                            nc.vector.tensor_copy(
                                ut_sb[:, h, ts(j, QC)], ot[h][:]
                            )
                        else:
                            li = rb_pool.tile([P, QC], F32, name="li", tag="li")
                            nc.vector.reciprocal(li[:], lb[h][:])
                            for sub in range(4):
                                nc.vector.tensor_mul(
                                    ut_sb[:, h, QC * j + P * sub : QC * j + P * (sub + 1)],
                                    ot[h][:, ts(sub, P)],
                                    li[:, ts(sub, P)],
                                )

                # ---- phase E: UT += Wself_h @ x.T ----
                for m in range(HPC if "E" in phases else 0):
                    for j in range(NQC):
                        ps = ps_a.tile([P, QC], F32, name="ps_e2", tag="ps_a")
                        for c in range(NDC):
                            nc.tensor.matmul(
                                ps[:],
                                wselfT_sb[:, c, ts(m, HD)],
                                xT_sb[:, c, ts(j, QC)],
                                start=(c == 0),
                                stop=(c == NDC - 1),
                            )
                        for sub in range(4):
                            sl = slice(QC * j + P * sub, QC * j + P * (sub + 1))
                            nc.vector.tensor_add(
                                ut_sb[:, m, sl],
                                ut_sb[:, m, sl],
                                ps[:, bass.ts(sub, P)],
                            )

                # ---- phase F: partial = U @ Wout.T slice ----
                for qt in range(NKT if "F" in phases else 0):
                    ob = out_pool.tile([P, D], BF16, name="ob", tag="ob")
                    for nch in range(2):
                        ps = ps_a.tile([P, QC], F32, name="ps_f", tag="ps_a")
                        for m in range(HPC):
                            nc.tensor.matmul(
                                ps[:],
                                ut_sb[:, m, ts(qt, P)],
                                woutT_sb[:, m, ts(nch, QC)],
                                start=(m == 0),
                                stop=(m == HPC - 1),
                            )
                        if nch == 0:
                            nc.scalar.copy(ob[:, ts(nch, QC)], ps[:])
                        else:
                            nc.vector.tensor_copy(ob[:, ts(nch, QC)], ps[:])
                    nc.sync.dma_start(part[ts(qt, P), :], ob[:])
                    if qt == NKT - 1:
                        nc.sync.dma_start(tick[:, :], ob[0:1, 0:8])

    _legalize_waits(nc)
    return nc


_NC = None


def _get_nc():
    global _NC
    if _NC is None:
        _NC = _build()
    return _NC


def build_in_maps(x, Wk, Wv, Wself, Wout):
    import ml_dtypes

    BF = ml_dtypes.bfloat16
    x = np.asarray(x, dtype=np.float32)
    Wk = np.asarray(Wk, dtype=np.float32)
    Wv = np.asarray(Wv, dtype=np.float32)
    Wself = np.asarray(Wself, dtype=np.float32)
    Wout = np.asarray(Wout, dtype=np.float32)

    kscale = np.float32(HD ** -0.25)
    xT = [np.ascontiguousarray(x[b].T).astype(BF) for b in range(B)]
    ones_m = np.ones((P, P), BF)
    mask01 = np.triu(np.ones((P, P), BF))  # (k,q): allow q >= k

    in_maps = []
    for c in range(N_CORES):
        b, hp = divmod(c, 4)
        ds = slice(DS * hp, DS * (hp + 1))
        in_maps.append(
            {
                "xT": xT[b],
                "wkT": np.ascontiguousarray((Wk[ds, :] * kscale).T).astype(BF),
                "wvT": np.ascontiguousarray(Wv[ds, :].T).astype(BF),
                "wselfT": np.ascontiguousarray(Wself[ds, :].T).astype(BF),
                "woutT": np.ascontiguousarray(Wout[:, ds].T).astype(BF),
                "ones_m": ones_m,
                "mask01": mask01,
            }
        )
    return in_maps


def kernel(x, Wk, Wv, Wself, Wout):
    nc = _get_nc()
    in_maps = build_in_maps(x, Wk, Wv, Wself, Wout)
    res = run_bass_kernel_spmd(nc, in_maps, core_ids=list(range(N_CORES)))

    out = np.empty((B, S, D), np.float32)
    for b in range(B):
        acc = np.zeros((S, D), np.float32)
        for hp in range(4):
            acc += np.asarray(res.results[4 * b + hp]["part"], np.float32)
        out[b] = acc
    return out


# revision 5
# speedup vs baseline: 3.3920x; 1.6560x over previous
"""Trainium2 Bass kernel for nn_EnergyFunction (dense transformer block).

Reference math (B=2, S=2048, D=1024, H=8 heads, hd=128):
    K  = x @ Wk.T            [B,S,D] -> heads [B,H,S,hd]
    V  = x @ Wv.T
    E  = (K K^T)/sqrt(hd)    per head, causal mask (q >= k allowed)
    P  = softmax(-E, axis=k)
    O  = P @ V               -> [B,S,D]
    out = (O + x @ Wself.T) @ Wout.T

Sharding (8 cores): core c -> batch b=c//4, head pair hp=c%4 (heads 2hp,2hp+1,
dims ds=[256*hp, 256*hp+256)).  Each core computes
    partial_c = (O_heads + x @ Wself.T[:,ds]) @ Wout.T[ds,:]   [S, D]
and the host sums the 4 partials per batch (row/column-parallel Wout split).

On-core layout trick: all attention tensors are kept "transposed"
(k or head-dim on partitions, q on free dim).  E is symmetric, so score
tiles are computed directly in (k-part, q-free) orientation by swapping
matmul operands -- no on-chip transposes are needed anywhere.  The softmax
denominator l_q = sum_k P[q,k] is taken with an all-ones [128,128] matmul
accumulated alongside the P@V matmuls, which lands 1/l's operand in PSUM
already broadcast across partitions (no cross-partition reduction or
transpose is ever needed).  Softmax max-subtraction
is skipped: |E|/sqrt(hd) <= ~11 for this distribution, exp() is safe in f32.

All operands (x, weights, K, V, P, U, partial output) are bf16: halves every
DMA and SBUF footprint, runs matmuls at full PE rate at ANY free size (f32r
needs free>=256), and keeps PSUM accumulation in f32.  rel-err budget is
2e-2; bf16 I/O lands ~2e-3.  The 1/sqrt(hd) scaling is folded into Wk on
the host (hd**-0.25 on both operands of K K^T).
"""

import os
import sys

import numpy as np

if "/opt/trn_rl_repo" not in sys.path:
    sys.path.insert(0, "/opt/trn_rl_repo")

import concourse.bass as bass
import concourse.mybir as mybir
import concourse.tile as tile
from concourse.bass import ts
from concourse.bass_utils import run_bass_kernel_spmd

B, S, D = 2, 2048, 1024
H = 8
HD = D // H          # 128 head dim
HPC = 2              # heads per core
DS = HPC * HD        # 256 dims per core
N_CORES = 8
P = 128              # partitions
QC = 512             # q chunk width
NQC = S // QC        # 4 q chunks
NKT = S // P         # 16 k tiles
NDC = D // P         # 8 contraction chunks over D

F32 = mybir.dt.float32
F32R = mybir.dt.float32r
BF16 = mybir.dt.bfloat16
EXP = mybir.ActivationFunctionType.Exp


def _legalize_waits(nc):
    """This toolchain's walrus rejects >1 semaphore wait on several
    instruction structs (Drain/CTRL allows none, Matmult/Ldweights S3_LW
    allows one).  Hoist excess waits onto same-engine NOPs placed
    immediately before the offending instruction."""
    for blk in nc.main_func.blocks:
        insts = blk.instructions
        new = []
        changed = False
        for ins in insts:
            si = ins.sync_info
            if si is not None and si.on_wait:
                allow = 0 if ins.opcode == "Drain" else 1
                waits = list(si.on_wait)
                if len(waits) > allow:
                    cut = len(waits) - allow
                    for k, w in enumerate(waits[:cut]):
                        nop = mybir.InstNoOp(
                            name=f"{ins.name}-wsplit{k}", engine=ins.engine
                        )
                        nop.sync_info = mybir.SyncInfo(on_wait=[w], on_update=[])
                        new.append(nop)
                    ins.sync_info = mybir.SyncInfo(
                        on_wait=waits[cut:], on_update=list(si.on_update)
                    )
                    changed = True
            new.append(ins)
        if changed:
            blk.instructions = new


def _build(repeats=1, loop_n=None, copy_eng="mix", skip_l=False, phases="BCDEF"):
    """loop_n: timing-only mode — wrap the body in a device-side For_i loop
    so NEFF execution time dominates the ~200 ms axon RPC floor.
    copy_eng/skip_l/phases: timing experiment knobs (skip_l and partial
    phases produce WRONG results — timing only)."""
    nc = bass.Bass()

    xT = nc.dram_tensor("xT", [D, S], BF16, kind="ExternalInput")
    wkT = nc.dram_tensor("wkT", [D, DS], BF16, kind="ExternalInput")
    wvT = nc.dram_tensor("wvT", [D, DS], BF16, kind="ExternalInput")
    wselfT = nc.dram_tensor("wselfT", [D, DS], BF16, kind="ExternalInput")
    woutT = nc.dram_tensor("woutT", [DS, D], BF16, kind="ExternalInput")
    ones_m = nc.dram_tensor("ones_m", [P, P], BF16, kind="ExternalInput")
    mask01 = nc.dram_tensor("mask01", [P, P], BF16, kind="ExternalInput")
    part = nc.dram_tensor("part", [S, D], BF16, kind="ExternalOutput")
    # tiny completion-marker output: lets timing harnesses wait for NEFF
    # completion without pulling the partial off the device
    tick = nc.dram_tensor("tick", [1, 8], BF16, kind="ExternalOutput")

    with tile.TileContext(nc) as tc:
        with (
            tc.tile_pool(name="persist", bufs=1) as pp,
            tc.tile_pool(name="pt_pool", bufs=8) as pt_pool,
            tc.tile_pool(name="rb_pool", bufs=2) as rb_pool,
            tc.tile_pool(name="out_pool", bufs=3) as out_pool,
            tc.tile_pool(name="ps_a", bufs=4, space="PSUM") as ps_a,
            tc.tile_pool(name="ps_ot", bufs=2, space="PSUM") as ps_ot,
            tc.tile_pool(name="ps_l", bufs=2, space="PSUM") as ps_l,
        ):
            # ---- persistent SBUF tensors ----
            xT_sb = pp.tile([P, NDC, S], BF16, name="xT_sb")
            wkT_sb = pp.tile([P, NDC, DS], BF16, name="wkT_sb")
            wvT_sb = pp.tile([P, NDC, DS], BF16, name="wvT_sb")
            wselfT_sb = pp.tile([P, NDC, DS], BF16, name="wselfT_sb")
            woutT_sb = pp.tile([P, HPC, D], BF16, name="woutT_sb")
            kt_sb = pp.tile([P, HPC, S], BF16, name="kt_sb")
            v_sb = pp.tile([P, NKT, DS], BF16, name="v_sb")
            ut_sb = pp.tile([P, HPC, S], BF16, name="ut_sb")
            onesm_sb = pp.tile([P, P], BF16, name="onesm_sb")
            mask_sb = pp.tile([P, P], BF16, name="mask_sb")

            def pcopy(dst, src_):
                # psum->sbuf copies: split halves across ACT+DVE so the
                # PSUM bank frees in half the latency
                if copy_eng == "act":
                    nc.scalar.copy(dst, src_)
                elif copy_eng == "dve":
                    nc.vector.tensor_copy(dst, src_)
                else:
                    w = dst.shape[-1]
                    h = w // 2
                    nc.scalar.copy(dst[:, :h], src_[:, :h])
                    nc.vector.tensor_copy(dst[:, h:], src_[:, h:])

            # ---- load small consts + weights (in first-use order;
            # wkT/x chunk loads are interleaved so phase B starts early)
            nc.sync.dma_start(onesm_sb[:], ones_m[:])
            nc.sync.dma_start(mask_sb[:], mask01[:])
            import contextlib

            loop_ctx = (
                tc.For_i(0, loop_n, 1) if loop_n else contextlib.nullcontext()
            )
            with loop_ctx:
              for _rep in range(repeats):
                for c in range(NDC):
                    nc.sync.dma_start(wkT_sb[:, c, :], wkT[ts(c, P), :])
                    nc.sync.dma_start(
                        xT_sb[:, c, 0:QC], xT[ts(c, P), 0:QC]
                    )
                for j in range(1, NQC):
                    for c in range(NDC):
                        nc.sync.dma_start(
                            xT_sb[:, c, ts(j, QC)], xT[ts(c, P), ts(j, QC)]
                        )
                    if j == 1:
                        for c in range(NDC):
                            nc.sync.dma_start(wvT_sb[:, c, :], wvT[ts(c, P), :])
                for c in range(NDC):
                    nc.sync.dma_start(wselfT_sb[:, c, :], wselfT[ts(c, P), :])
                for c in range(HPC):
                    nc.sync.dma_start(woutT_sb[:, c, :], woutT[ts(c, P), :])

                # ---- phases B+C interleaved per q-chunk: K^T and V for
                # chunk j are computed as soon as x chunk j lands, so the PE
                # has ~3x the work per x-chunk DMA and never drains ----
                for j in range(NQC):
                    for h in range(HPC if "B" in phases else 0):
                        ps = ps_a.tile([P, QC], F32, name="ps_b", tag="ps_a")
                        for c in range(NDC):
                            nc.tensor.matmul(
                                ps[:],
                                wkT_sb[:, c, ts(h, HD)],
                                xT_sb[:, c, ts(j, QC)],
                                start=(c == 0),
                                stop=(c == NDC - 1),
                            )
                        pcopy(kt_sb[:, h, ts(j, QC)], ps[:])

                    for st in range(4 * j, 4 * j + 4) if "C" in phases else []:
                        ps = ps_a.tile([P, QC], F32, name="ps_c", tag="ps_a")
                        for c in range(NDC):
                            nc.tensor.matmul(
                                ps[:, :DS],
                                xT_sb[:, c, ts(st, P)],
                                wvT_sb[:, c, :],
                                start=(c == 0),
                                stop=(c == NDC - 1),
                            )
                        pcopy(v_sb[:, st, :], ps[:, :DS])

                # ---- phase D: attention, heads interleaved per q-chunk ----
                # L matmul uses an all-ones [128,128] lhsT so the softmax
                # denominator lands in PSUM already broadcast across
                # partitions: PE never has to wait on DVE in this phase.
                # Software-pipelined: the PV/L matmuls for k-tile kt issue
                # AFTER the score matmuls for k-tile kt+1, so the
                # PSUM-drain -> exp(ACT) -> mask(DVE) chain of tile kt hides
                # behind ~426ns of independent PE work (PE executes in
                # program order; without the reorder that latency is exposed
                # on every (kt,h) pair).
                for j in range(NQC if "D" in phases else 0):
                    nkt = 4 * j + 4  # causal: k tiles 0..4j+3
                    ot = {}
                    lb = {}
                    for h in range(HPC):
                        ot[h] = ps_ot.tile(
                            [P, QC], F32, name=f"ot{h}", tag="ps_ot"
                        )
                        lb[h] = ps_l.tile(
                            [P, QC], F32, name=f"lb{h}", tag="ps_l"
                        )

                    def flush(pend, j=j, nkt=nkt, ot=ot, lb=lb):
                        kt, c0, pts = pend
                        for h in range(HPC):
                            nc.tensor.matmul(
                                ot[h][:, c0:],
                                v_sb[:, kt, ts(h, HD)],
                                pts[h][:, c0:],
                                start=(kt == 0),
                                stop=(kt == nkt - 1),
                            )
                            if not skip_l:
                                nc.tensor.matmul(
                                    lb[h][:, c0:],
                                    onesm_sb[:],
                                    pts[h][:, c0:],
                                    start=(kt == 0),
                                    stop=(kt == nkt - 1),
                                )

                    pend = None
                    for kt in range(nkt):
                        c0 = max(0, P * kt - QC * j)
                        pts = {}
                        for h in range(HPC):
                            ep = ps_a.tile([P, QC], F32, name="ep", tag="ps_a")
                            # scores (k-part, q-free): E^T = KT[kt].T @ KT[qch]
                            nc.tensor.matmul(
                                ep[:, c0:],
                                kt_sb[:, h, ts(kt, P)],
                                kt_sb[:, h, QC * j + c0 : QC * (j + 1)],
                                start=True,
                                stop=True,
                            )
                            pt = pt_pool.tile([P, QC], BF16, name="pt", tag="pt")
                            nc.scalar.activation(
                                pt[:, c0:], ep[:, c0:], EXP, scale=-1.0
                            )
                            if kt >= 4 * j:
                                # diagonal subtile: zero disallowed (q < k)
                                nc.vector.tensor_mul(
                                    pt[:, c0 : c0 + P],
                                    pt[:, c0 : c0 + P],
                                    mask_sb[:],
                                )
                            pts[h] = pt
                        if pend is not None:
                            flush(pend)
                        pend = (kt, c0, pts)
                    flush(pend)
                    for h in range(HPC):
                        if skip_l:
                            nc.vector.tensor_copy(
                                ut_sb[:, h, ts(j, QC)], ot[h][:]
                            )
                        else:
                            li = rb_pool.tile([P, QC], F32, name="li", tag="li")
                            nc.vector.reciprocal(li[:], lb[h][:])
                            nc.vector.tensor_mul(
                                ut_sb[:, h, ts(j, QC)], ot[h][:], li[:]
                            )

                # ---- phase E: UT += Wself_h @ x.T ----
                for m in range(HPC if "E" in phases else 0):
                    for j in range(NQC):
                        ps = ps_a.tile([P, QC], F32, name="ps_e2", tag="ps_a")
                        for c in range(NDC):
                            nc.tensor.matmul(
                                ps[:],
                                wselfT_sb[:, c, ts(m, HD)],
                                xT_sb[:, c, ts(j, QC)],
                                start=(c == 0),
                                stop=(c == NDC - 1),
                            )
                        nc.vector.tensor_add(
                            ut_sb[:, m, ts(j, QC)],
                            ut_sb[:, m, ts(j, QC)],
                            ps[:],
                        )

                # ---- phase F: partial = U @ Wout.T slice ----
                for qt in range(NKT if "F" in phases else 0):
                    ob = out_pool.tile([P, D], BF16, name="ob", tag="ob")
                    for nch in range(2):
                        ps = ps_a.tile([P, QC], F32, name="ps_f", tag="ps_a")
                        for m in range(HPC):
                            nc.tensor.matmul(
                                ps[:],
                                ut_sb[:, m, ts(qt, P)],
                                woutT_sb[:, m, ts(nch, QC)],
                                start=(m == 0),
                                stop=(m == HPC - 1),
                            )
                        if nch == 0:
                            nc.scalar.copy(ob[:, ts(nch, QC)], ps[:])
                        else:
                            nc.vector.tensor_copy(ob[:, ts(nch, QC)], ps[:])
                    nc.sync.dma_start(part[ts(qt, P), :], ob[:])
                    if qt == NKT - 1:
                        nc.sync.dma_start(tick[:, :], ob[0:1, 0:8])

    _legalize_waits(nc)
    return nc


_NC = None


def _get_nc():
    global _NC
    if _NC is None:
        _NC = _build()
    return _NC


def build_in_maps(x, Wk, Wv, Wself, Wout):
    import ml_dtypes

    BF = ml_dtypes.bfloat16
    x = np.asarray(x, dtype=np.float32)
    Wk = np.asarray(Wk, dtype=np.float32)
    Wv = np.asarray(Wv, dtype=np.float32)
    Wself = np.asarray(Wself, dtype=np.float32)
    Wout = np.asarray(Wout, dtype=np.float32)

    kscale = np.float32(HD ** -0.25)
    xT = [np.ascontiguousarray(x[b].T).astype(BF) for b in range(B)]
    ones_m = np.ones((P, P), BF)
    mask01 = np.triu(np.ones((P, P), BF))  # (k,q): allow q >= k

    in_maps = []
    for c in range(N_CORES):
        b, hp = divmod(c, 4)
        ds = slice(DS * hp, DS * (hp + 1))
        in_maps.append(
            {
                "xT": xT[b],
                "wkT": np.ascontiguousarray((Wk[ds, :] * kscale).T).astype(BF),
                "wvT": np.ascontiguousarray(Wv[ds, :].T).astype(BF),
                "wselfT": np.ascontiguousarray(Wself[ds, :].T).astype(BF),
                "woutT": np.ascontiguousarray(Wout[:, ds].T).astype(BF),
                "ones_m": ones_m,
                "mask01": mask01,
            }
        )
    return in_maps


def kernel(x, Wk, Wv, Wself, Wout):
    nc = _get_nc()
    in_maps = build_in_maps(x, Wk, Wv, Wself, Wout)
    res = run_bass_kernel_spmd(nc, in_maps, core_ids=list(range(N_CORES)))

    out = np.empty((B, S, D), np.float32)
    for b in range(B):
        acc = np.zeros((S, D), np.float32)
        for hp in range(4):
            acc += np.asarray(res.results[4 * b + hp]["part"], np.float32)
        out[b] = acc
    return out
